# revision 5
# baseline (speedup 1.0000x reference)
"""Trainium2 Bass kernel: 57-bit barrel shift right with sticky bit.

Contract: kernel(X, shift) takes the FULL inputs
  X     [1_000_000, 57] float32  (0/1 bit values)
  shift [1_000_000, 6]  float32  (0/1 bits, MSB first)
returns (result [1_000_000, 57] f32, sticky [1_000_000, 1] f32),
matching reference._barrel_shift_right_sticky.

Strategy: pure data parallel over 8 NeuronCores (125k examples each,
padded to 125,056 = 977*128 rows per core).

Per-core algorithm ("radix-256"): each example's 57 bits are DMA-cast
f32 -> u8 into 64 data bytes of a 96-byte group = 24 u32 words
(words 0..7 zero pad, words 8..23 = 64 data bytes; bytes 57..63 zero).
Right-shifting the bit array by s == moving bytes to higher indices ==
multi-word LEFT shift by s byte positions. s = 4*w + r2: r2 (0..3) is
done with one variable-shift stage (<< 8*r2 with cross-word carries),
w (0..15) with four copy_predicated mux layers (1/2/4/8 words) whose
masks are the shift bits themselves. Sticky = OR of dropped bits =
per-layer register carry-outs (masked) OR bytes 57..63 of the final
value. All bytes stay exactly 0/1 so the final u8 -> f32 DMA cast
reproduces exact 0.0/1.0 outputs.
"""
import numpy as np

import concourse.bacc as bacc
import concourse.mybir as mybir
import concourse.tile as tile
from concourse.bass_utils import run_bass_kernel_spmd

N_CORES = 8
DATA_BITS = 57
SHIFT_BITS = 6
B_TOTAL = 1_000_000
B_CORE = B_TOTAL // N_CORES          # 125_000
P = 128
B_PAD = 977 * P                      # 125_056 rows per core (zero padded)
TILE_PLAN = [(i * 128 * P, 128) for i in range(7)] + [(7 * 128 * P, 81)]

NW = 16          # data words per example
NWP = 24         # padded words per example (8 pad + 16 data)
DATA_W0 = 8      # first data word index
DATA_B0 = DATA_W0 * 4


def _barrel_tile(nc, pool, x_rows, sh_rows, out_rows, stk_rows, C):
    dt = mybir.dt
    op = mybir.AluOpType
    A = pool.tile([P, C, NWP], dt.uint32, tag="A")
    B = pool.tile([P, C, NWP], dt.uint32, tag="B")
    t1 = pool.tile([P, C, NW], dt.uint32, tag="t1")
    t2 = pool.tile([P, C, NW], dt.uint32, tag="t2")
    shf = pool.tile([P, C, SHIFT_BITS], dt.float32, tag="shf")

    nc.vector.memset(A[:], 0)
    nc.vector.memset(B[:, :, 0:DATA_W0], 0)

    A8 = A.bitcast(dt.uint8)   # [P, C, 96]

    # split loads to stay under the 16384-descriptor cap (128*C 57B runs)
    Ch = (C + 1) // 2
    nc.gpsimd.dma_start(out=A8[:, 0:Ch, DATA_B0:DATA_B0 + DATA_BITS],
                        in_=x_rows[:, 0:Ch, :])
    nc.gpsimd.dma_start(out=A8[:, Ch:C, DATA_B0:DATA_B0 + DATA_BITS],
                        in_=x_rows[:, Ch:C, :])
    nc.sync.dma_start(out=shf[:], in_=sh_rows)

    # shift-amount pieces (f32 domain, then cast to u32)
    r8f = pool.tile([P, C], dt.float32, tag="r8f")      # 8*r2 = 16*c4 + 8*c5
    invf = pool.tile([P, C], dt.float32, tag="invf")    # 31 - 8*r2
    r8 = pool.tile([P, C], dt.uint32, tag="r8")
    inv = pool.tile([P, C], dt.uint32, tag="inv")
    c5x8 = pool.tile([P, C], dt.float32, tag="c5x8")
    nc.vector.tensor_scalar(out=r8f[:], in0=shf[:, :, 4:5].squeeze(2),
                            scalar1=16.0, scalar2=None, op0=op.mult)
    nc.vector.tensor_scalar(out=c5x8[:], in0=shf[:, :, 5:6].squeeze(2),
                            scalar1=8.0, scalar2=None, op0=op.mult)
    nc.vector.tensor_tensor(out=r8f[:], in0=r8f[:], in1=c5x8[:], op=op.add)
    nc.vector.tensor_scalar(out=invf[:], in0=r8f[:], scalar1=-1.0, scalar2=31.0,
                            op0=op.mult, op1=op.add)
    nc.vector.tensor_copy(r8[:], r8f[:])
    nc.vector.tensor_copy(inv[:], invf[:])

    # integer (u8) copy of the shift bits: walrus requires int mask dtype
    shfi = pool.tile([P, C, SHIFT_BITS], dt.uint8, tag="shfi")
    nc.vector.tensor_copy(shfi[:], shf[:])

    acc = pool.tile([P, C], dt.uint32, tag="acc")  # sticky accumulator

    # variable stage: << 8*r2 bits with carries, A -> B
    r8b = r8[:].unsqueeze(2).to_broadcast([P, C, NW])
    invb = inv[:].unsqueeze(2).to_broadcast([P, C, NW])
    Ad = A[:, :, DATA_W0:NWP]
    Apv = A[:, :, DATA_W0 - 1:NWP - 1]   # prev word (word 7 = pad zero)
    nc.vector.tensor_tensor(out=t1[:], in0=Ad, in1=r8b, op=op.logical_shift_left)
    nc.vector.tensor_tensor(out=t2[:], in0=Apv, in1=invb,
                            op=op.logical_shift_right)
    nc.vector.tensor_scalar(out=t2[:], in0=t2[:], scalar1=1, scalar2=None,
                            op0=op.logical_shift_right)
    nc.vector.tensor_tensor(out=B[:, :, DATA_W0:NWP], in0=t1[:], in1=t2[:],
                            op=op.bitwise_or)
    co = pool.tile([P, C], dt.uint32, tag="co")
    nc.vector.tensor_tensor(out=co[:], in0=A[:, :, NWP - 1], in1=inv[:],
                            op=op.logical_shift_right)
    nc.vector.tensor_scalar(out=acc[:], in0=co[:], scalar1=1, scalar2=None,
                            op0=op.logical_shift_right)

    # word-shift mux layers
    src, dst = B, A
    for (amt, col) in ((1, 3), (2, 2), (4, 1), (8, 0)):
        mask1 = shfi[:, :, col:col + 1].squeeze(2)
        maskb = shfi[:, :, col:col + 1].to_broadcast([P, C, NW])
        cap = pool.tile([P, C], dt.uint32, tag="cap")
        if amt == 1:
            nc.vector.tensor_copy(cap[:], src[:, :, NWP - 1])
        else:
            nc.vector.tensor_reduce(out=cap[:], in_=src[:, :, NWP - amt:NWP],
                                    axis=mybir.AxisListType.X, op=op.max)
        tmax = pool.tile([P, C], dt.uint32, tag="tmax")
        nc.vector.tensor_tensor(out=tmax[:], in0=acc[:], in1=cap[:], op=op.max)
        nc.vector.copy_predicated(out=acc[:], mask=mask1, data=tmax[:])
        nc.vector.tensor_copy(dst[:, :, DATA_W0:NWP], src[:, :, DATA_W0:NWP])
        nc.vector.copy_predicated(out=dst[:, :, DATA_W0:NWP], mask=maskb,
                                  data=src[:, :, DATA_W0 - amt:NWP - amt])
        src, dst = dst, src

    fin = src
    fin8 = fin.bitcast(dt.uint8)

    # final sticky: bytes 57..63 of the final value OR carry accum
    tl = pool.tile([P, C], dt.uint32, tag="tl")
    nc.vector.tensor_scalar(out=tl[:], in0=fin[:, :, NWP - 2], scalar1=8,
                            scalar2=None, op0=op.logical_shift_right)
    nc.vector.tensor_tensor(out=tl[:], in0=tl[:], in1=fin[:, :, NWP - 1],
                            op=op.max)
    nc.vector.tensor_tensor(out=tl[:], in0=tl[:], in1=acc[:], op=op.max)
    stk = pool.tile([P, C], dt.float32, tag="stk")
    nc.vector.tensor_scalar(out=stk[:], in0=tl[:], scalar1=0, scalar2=None,
                            op0=op.not_equal)

    nc.gpsimd.dma_start(out=out_rows[:, 0:Ch, :],
                        in_=fin8[:, 0:Ch, DATA_B0:DATA_B0 + DATA_BITS])
    nc.gpsimd.dma_start(out=out_rows[:, Ch:C, :],
                        in_=fin8[:, Ch:C, DATA_B0:DATA_B0 + DATA_BITS])
    nc.sync.dma_start(out=stk_rows, in_=stk[:].unsqueeze(2))


def _build_program():
    nc = bacc.Bacc("TRN2", target_bir_lowering=False, debug=False)
    x = nc.dram_tensor("X", [B_PAD, DATA_BITS], mybir.dt.float32,
                       kind="ExternalInput").ap()
    sh = nc.dram_tensor("shift", [B_PAD, SHIFT_BITS], mybir.dt.float32,
                        kind="ExternalInput").ap()
    out = nc.dram_tensor("out", [B_PAD, DATA_BITS], mybir.dt.float32,
                         kind="ExternalOutput").ap()
    stk = nc.dram_tensor("stk", [B_PAD, 1], mybir.dt.float32,
                         kind="ExternalOutput").ap()
    with tile.TileContext(nc) as tc:
        with tc.tile_pool(name="pool", bufs=2) as pool:
            for (b0, C) in TILE_PLAN:
                n = P * C
                xr = x[b0:b0 + n, :].rearrange("(p c) j -> p c j", c=C)
                sr = sh[b0:b0 + n, :].rearrange("(p c) j -> p c j", c=C)
                orr = out[b0:b0 + n, :].rearrange("(p c) j -> p c j", c=C)
                kr = stk[b0:b0 + n, :].rearrange("(p c) j -> p c j", c=C)
                _barrel_tile(nc, pool, xr, sr, orr, kr, C)
    nc.compile()
    return nc


_PROGRAM = None


def _get_program():
    global _PROGRAM
    if _PROGRAM is None:
        _PROGRAM = _build_program()
    return _PROGRAM


def run(X, shift, trace=False):
    """Run on 8 NeuronCores. Returns ((result, sticky), BassKernelResults)."""
    X = np.ascontiguousarray(np.asarray(X, dtype=np.float32))
    shift = np.ascontiguousarray(np.asarray(shift, dtype=np.float32))
    assert X.shape == (B_TOTAL, DATA_BITS), X.shape
    assert shift.shape == (B_TOTAL, SHIFT_BITS), shift.shape
    nc = _get_program()
    in_maps = []
    for i in range(N_CORES):
        xp = np.zeros((B_PAD, DATA_BITS), np.float32)
        sp = np.zeros((B_PAD, SHIFT_BITS), np.float32)
        xp[:B_CORE] = X[i * B_CORE:(i + 1) * B_CORE]
        sp[:B_CORE] = shift[i * B_CORE:(i + 1) * B_CORE]
        in_maps.append({"X": xp, "shift": sp})
    res = run_bass_kernel_spmd(nc, in_maps, core_ids=list(range(N_CORES)),
                               trace=trace)
    outs = np.concatenate([r["out"][:B_CORE] for r in res.results], axis=0)
    stks = np.concatenate([r["stk"][:B_CORE] for r in res.results], axis=0)
    return (outs, stks), res


def kernel(X, shift):
    (outs, stks), _ = run(X, shift)
    return outs, stks


# revision 8
# speedup vs baseline: 2.5545x; 2.5545x over previous
"""Trainium2 Bass kernel: 57-bit barrel shift right with sticky bit.

Contract: kernel(X, shift) takes the FULL inputs
  X     [1_000_000, 57] float32  (0/1 bit values)
  shift [1_000_000, 6]  float32  (0/1 bits, MSB first)
returns (result [1_000_000, 57] f32, sticky [1_000_000, 1] f32),
matching reference._barrel_shift_right_sticky.

Strategy: pure data parallel over 8 NeuronCores (125k examples each,
padded to 125,056 = 977*128 rows per core).

Per-core algorithm ("radix-256"): each example's 57 bits are DMA-cast
f32 -> u8 into 64 data bytes of a 96-byte group = 24 u32 words
(words 0..7 zero pad, words 8..23 = 64 data bytes; bytes 57..63 zero).
Right-shifting the bit array by s == moving bytes to higher indices ==
multi-word LEFT shift by s byte positions. s = 4*w + r2: r2 (0..3) is
done with one variable-shift stage (<< 8*r2 with cross-word carries),
w (0..15) with four copy_predicated mux layers (1/2/4/8 words) whose
masks are the shift bits themselves. Sticky = OR of dropped bits =
per-layer register carry-outs (masked) OR bytes 57..63 of the final
value. All bytes stay exactly 0/1 so the final u8 -> f32 DMA cast
reproduces exact 0.0/1.0 outputs.
"""
import numpy as np

import concourse.bacc as bacc
import concourse.mybir as mybir
import concourse.tile as tile
from concourse.bass_utils import run_bass_kernel_spmd

N_CORES = 8
DATA_BITS = 57
SHIFT_BITS = 6
B_TOTAL = 1_000_000
B_CORE = B_TOTAL // N_CORES          # 125_000
P = 128
B_PAD = 977 * P                      # 125_056 rows per core (zero padded)
TILE_PLAN = [(i * 128 * P, 128) for i in range(7)] + [(7 * 128 * P, 81)]

NW = 16          # data words per example
NWP = 24         # padded words per example (8 pad + 16 data)
DATA_W0 = 8      # first data word index
DATA_B0 = DATA_W0 * 4


def _barrel_tile(nc, pool, pool1, x_rows, sh_rows, out_rows, stk_rows, C):
    dt = mybir.dt
    op = mybir.AluOpType
    A = pool1.tile([P, C, NWP], dt.uint32, tag="A")
    B = pool1.tile([P, C, NWP], dt.uint32, tag="B")
    t1 = pool1.tile([P, C, NW], dt.uint32, tag="t1")
    t2 = pool1.tile([P, C, NW], dt.uint32, tag="t2")
    shf = pool.tile([P, C, SHIFT_BITS], dt.float32, tag="shf")
    xf = pool.tile([P, C, DATA_BITS], dt.float32, tag="xf")

    nc.scalar.memzero(A[:])
    nc.vector.memset(B[:, :, 0:DATA_W0], 0)

    A8 = A.bitcast(dt.uint8)   # [P, C, 96]

    # contiguous HWDGE loads (128 big descriptors), cast f32->u8 on ACT
    nc.sync.dma_start(out=xf[:], in_=x_rows)
    nc.sync.dma_start(out=shf[:], in_=sh_rows)
    nc.scalar.copy(out=A8[:, :, DATA_B0:DATA_B0 + DATA_BITS], in_=xf[:])

    # shift-amount pieces (f32 domain, then cast to u32)
    r8f = pool.tile([P, C], dt.float32, tag="r8f")      # 8*r2 = 16*c4 + 8*c5
    invf = pool.tile([P, C], dt.float32, tag="invf")    # 31 - 8*r2
    r8 = pool.tile([P, C], dt.uint32, tag="r8")
    inv = pool.tile([P, C], dt.uint32, tag="inv")
    c5x8 = pool.tile([P, C], dt.float32, tag="c5x8")
    nc.vector.tensor_scalar(out=r8f[:], in0=shf[:, :, 4:5].squeeze(2),
                            scalar1=16.0, scalar2=None, op0=op.mult)
    nc.vector.tensor_scalar(out=c5x8[:], in0=shf[:, :, 5:6].squeeze(2),
                            scalar1=8.0, scalar2=None, op0=op.mult)
    nc.vector.tensor_tensor(out=r8f[:], in0=r8f[:], in1=c5x8[:], op=op.add)
    nc.vector.tensor_scalar(out=invf[:], in0=r8f[:], scalar1=-1.0, scalar2=31.0,
                            op0=op.mult, op1=op.add)
    nc.vector.tensor_copy(r8[:], r8f[:])
    nc.vector.tensor_copy(inv[:], invf[:])

    # integer (u8) copy of the shift bits: walrus requires int mask dtype
    shfi = pool.tile([P, C, SHIFT_BITS], dt.uint8, tag="shfi")
    nc.vector.tensor_copy(shfi[:], shf[:])

    acc = pool.tile([P, C], dt.uint32, tag="acc")  # sticky accumulator

    # variable stage: << 8*r2 bits with carries, A -> B
    r8b = r8[:].unsqueeze(2).to_broadcast([P, C, NW])
    invb = inv[:].unsqueeze(2).to_broadcast([P, C, NW])
    Ad = A[:, :, DATA_W0:NWP]
    Apv = A[:, :, DATA_W0 - 1:NWP - 1]   # prev word (word 7 = pad zero)
    nc.vector.tensor_tensor(out=t1[:], in0=Ad, in1=r8b, op=op.logical_shift_left)
    nc.vector.tensor_tensor(out=t2[:], in0=Apv, in1=invb,
                            op=op.logical_shift_right)
    nc.vector.tensor_scalar(out=t2[:], in0=t2[:], scalar1=1, scalar2=None,
                            op0=op.logical_shift_right)
    nc.vector.tensor_tensor(out=B[:, :, DATA_W0:NWP], in0=t1[:], in1=t2[:],
                            op=op.bitwise_or)
    co = pool.tile([P, C], dt.uint32, tag="co")
    nc.vector.tensor_tensor(out=co[:], in0=A[:, :, NWP - 1], in1=inv[:],
                            op=op.logical_shift_right)
    nc.vector.tensor_scalar(out=acc[:], in0=co[:], scalar1=1, scalar2=None,
                            op0=op.logical_shift_right)

    # word-shift mux layers
    src, dst = B, A
    for (amt, col) in ((1, 3), (2, 2), (4, 1), (8, 0)):
        mask1 = shfi[:, :, col:col + 1].squeeze(2)
        maskb = shfi[:, :, col:col + 1].to_broadcast([P, C, NW])
        cap = pool.tile([P, C], dt.uint32, tag="cap")
        if amt == 1:
            nc.vector.tensor_copy(cap[:], src[:, :, NWP - 1])
        else:
            nc.vector.tensor_reduce(out=cap[:], in_=src[:, :, NWP - amt:NWP],
                                    axis=mybir.AxisListType.X, op=op.max)
        tmax = pool.tile([P, C], dt.uint32, tag="tmax")
        nc.vector.tensor_tensor(out=tmax[:], in0=acc[:], in1=cap[:], op=op.max)
        nc.vector.copy_predicated(out=acc[:], mask=mask1, data=tmax[:])
        nc.vector.tensor_copy(dst[:, :, DATA_W0:NWP], src[:, :, DATA_W0:NWP])
        nc.vector.copy_predicated(out=dst[:, :, DATA_W0:NWP], mask=maskb,
                                  data=src[:, :, DATA_W0 - amt:NWP - amt])
        src, dst = dst, src

    fin = src
    fin8 = fin.bitcast(dt.uint8)

    # final sticky: bytes 57..63 of the final value OR carry accum
    tl = pool.tile([P, C], dt.uint32, tag="tl")
    nc.vector.tensor_scalar(out=tl[:], in0=fin[:, :, NWP - 2], scalar1=8,
                            scalar2=None, op0=op.logical_shift_right)
    nc.vector.tensor_tensor(out=tl[:], in0=tl[:], in1=fin[:, :, NWP - 1],
                            op=op.max)
    nc.vector.tensor_tensor(out=tl[:], in0=tl[:], in1=acc[:], op=op.max)
    stk = pool.tile([P, C], dt.float32, tag="stk")
    nc.vector.tensor_scalar(out=stk[:], in0=tl[:], scalar1=0, scalar2=None,
                            op0=op.not_equal)

    # cast u8->f32 on ACT, contiguous HWDGE store
    yf = pool.tile([P, C, DATA_BITS], dt.float32, tag="yf")
    nc.scalar.copy(out=yf[:], in_=fin8[:, :, DATA_B0:DATA_B0 + DATA_BITS])
    nc.sync.dma_start(out=out_rows, in_=yf[:])
    nc.sync.dma_start(out=stk_rows, in_=stk[:].unsqueeze(2))


def _build_program():
    nc = bacc.Bacc("TRN2", target_bir_lowering=False, debug=False)
    x = nc.dram_tensor("X", [B_PAD, DATA_BITS], mybir.dt.float32,
                       kind="ExternalInput").ap()
    sh = nc.dram_tensor("shift", [B_PAD, SHIFT_BITS], mybir.dt.float32,
                        kind="ExternalInput").ap()
    out = nc.dram_tensor("out", [B_PAD, DATA_BITS], mybir.dt.float32,
                         kind="ExternalOutput").ap()
    stk = nc.dram_tensor("stk", [B_PAD, 1], mybir.dt.float32,
                         kind="ExternalOutput").ap()
    with tile.TileContext(nc) as tc:
        with tc.tile_pool(name="pool", bufs=2) as pool, \
             tc.tile_pool(name="pool1", bufs=1) as pool1:
            for (b0, C) in TILE_PLAN:
                n = P * C
                xr = x[b0:b0 + n, :].rearrange("(p c) j -> p c j", c=C)
                sr = sh[b0:b0 + n, :].rearrange("(p c) j -> p c j", c=C)
                orr = out[b0:b0 + n, :].rearrange("(p c) j -> p c j", c=C)
                kr = stk[b0:b0 + n, :].rearrange("(p c) j -> p c j", c=C)
                _barrel_tile(nc, pool, pool1, xr, sr, orr, kr, C)
    nc.compile()
    return nc


_PROGRAM = None


def _get_program():
    global _PROGRAM
    if _PROGRAM is None:
        _PROGRAM = _build_program()
    return _PROGRAM


def run(X, shift, trace=False):
    """Run on 8 NeuronCores. Returns ((result, sticky), BassKernelResults)."""
    X = np.ascontiguousarray(np.asarray(X, dtype=np.float32))
    shift = np.ascontiguousarray(np.asarray(shift, dtype=np.float32))
    assert X.shape == (B_TOTAL, DATA_BITS), X.shape
    assert shift.shape == (B_TOTAL, SHIFT_BITS), shift.shape
    nc = _get_program()
    in_maps = []
    for i in range(N_CORES):
        xp = np.zeros((B_PAD, DATA_BITS), np.float32)
        sp = np.zeros((B_PAD, SHIFT_BITS), np.float32)
        xp[:B_CORE] = X[i * B_CORE:(i + 1) * B_CORE]
        sp[:B_CORE] = shift[i * B_CORE:(i + 1) * B_CORE]
        in_maps.append({"X": xp, "shift": sp})
    res = run_bass_kernel_spmd(nc, in_maps, core_ids=list(range(N_CORES)),
                               trace=trace)
    outs = np.concatenate([r["out"][:B_CORE] for r in res.results], axis=0)
    stks = np.concatenate([r["stk"][:B_CORE] for r in res.results], axis=0)
    return (outs, stks), res


def kernel(X, shift):
    (outs, stks), _ = run(X, shift)
    return outs, stks


# revision 11
# speedup vs baseline: 3.8585x; 1.5105x over previous
"""Trainium2 Bass kernel: 57-bit barrel shift right with sticky bit.

Contract: kernel(X, shift) takes the FULL inputs
  X     [1_000_000, 57] float32  (0/1 bit values)
  shift [1_000_000, 6]  float32  (0/1 bits, MSB first)
returns (result [1_000_000, 57] f32, sticky [1_000_000, 1] f32),
matching reference._barrel_shift_right_sticky.

Strategy: pure data parallel over 8 NeuronCores (125k examples each,
padded to 125,056 = 977*128 rows per core).

Per-core algorithm ("radix-256"): each example's 57 bits are DMA-cast
f32 -> u8 into 64 data bytes of a 96-byte group = 24 u32 words
(words 0..7 zero pad, words 8..23 = 64 data bytes; bytes 57..63 zero).
Right-shifting the bit array by s == moving bytes to higher indices ==
multi-word LEFT shift by s byte positions. s = 4*w + r2: r2 (0..3) is
done with one variable-shift stage (<< 8*r2 with cross-word carries),
w (0..15) with four copy_predicated mux layers (1/2/4/8 words) whose
masks are the shift bits themselves. Sticky = OR of dropped bits =
per-layer register carry-outs (masked) OR bytes 57..63 of the final
value. All bytes stay exactly 0/1 so the final u8 -> f32 DMA cast
reproduces exact 0.0/1.0 outputs.
"""
import numpy as np

import concourse.bacc as bacc
import concourse.mybir as mybir
import concourse.tile as tile
from concourse.bass_utils import run_bass_kernel_spmd

N_CORES = 8
DATA_BITS = 57
SHIFT_BITS = 6
B_TOTAL = 1_000_000
B_CORE = B_TOTAL // N_CORES          # 125_000
P = 128
B_PAD = 977 * P                      # 125_056 rows per core (zero padded)
TILE_PLAN = [(i * 112 * P, 112) for i in range(8)] + [(8 * 112 * P, 81)]

NW = 16          # data words per example
NWP = 24         # padded words per example (8 pad + 16 data)
DATA_W0 = 8      # first data word index
DATA_B0 = DATA_W0 * 4


def _barrel_tile(nc, pool, pool1, x_rows, sh_rows, out_rows, stk_rows, C):
    dt = mybir.dt
    op = mybir.AluOpType
    A = pool.tile([P, C, NWP], dt.uint32, tag="A")
    B = pool.tile([P, C, NWP], dt.uint32, tag="B")
    t1 = pool.tile([P, C, NW], dt.uint32, tag="t1")
    t2 = pool.tile([P, C, NW], dt.uint32, tag="t2")
    shf = pool.tile([P, C, SHIFT_BITS], dt.float32, tag="shf")
    xf = pool.tile([P, C, DATA_BITS], dt.float32, tag="xf")

    nc.scalar.memzero(A[:])
    nc.vector.memset(B[:, :, 0:DATA_W0], 0)

    A8 = A.bitcast(dt.uint8)   # [P, C, 96]

    # contiguous HWDGE loads (128 big descriptors), cast f32->u8 on ACT
    nc.sync.dma_start(out=xf[:], in_=x_rows)
    nc.sync.dma_start(out=shf[:], in_=sh_rows)
    nc.scalar.copy(out=A8[:, :, DATA_B0:DATA_B0 + DATA_BITS], in_=xf[:])

    # shift-amount pieces (f32 domain, then cast to u32)
    r8f = pool.tile([P, C], dt.float32, tag="r8f")      # 8*r2 = 16*c4 + 8*c5
    invf = pool.tile([P, C], dt.float32, tag="invf")    # 31 - 8*r2
    r8 = pool.tile([P, C], dt.uint32, tag="r8")
    inv = pool.tile([P, C], dt.uint32, tag="inv")
    c5x8 = pool.tile([P, C], dt.float32, tag="c5x8")
    nc.vector.tensor_scalar(out=r8f[:], in0=shf[:, :, 4:5].squeeze(2),
                            scalar1=16.0, scalar2=None, op0=op.mult)
    nc.vector.tensor_scalar(out=c5x8[:], in0=shf[:, :, 5:6].squeeze(2),
                            scalar1=8.0, scalar2=None, op0=op.mult)
    nc.vector.tensor_tensor(out=r8f[:], in0=r8f[:], in1=c5x8[:], op=op.add)
    nc.vector.tensor_scalar(out=invf[:], in0=r8f[:], scalar1=-1.0, scalar2=31.0,
                            op0=op.mult, op1=op.add)
    nc.vector.tensor_copy(r8[:], r8f[:])
    nc.vector.tensor_copy(inv[:], invf[:])

    # integer (u8) copy of the shift bits: walrus requires int mask dtype
    shfi = pool.tile([P, C, SHIFT_BITS], dt.uint8, tag="shfi")
    nc.vector.tensor_copy(shfi[:], shf[:])

    acc = pool.tile([P, C], dt.uint32, tag="acc")  # sticky accumulator

    # variable stage: << 8*r2 bits with carries, A -> B
    r8b = r8[:].unsqueeze(2).to_broadcast([P, C, NW])
    invb = inv[:].unsqueeze(2).to_broadcast([P, C, NW])
    Ad = A[:, :, DATA_W0:NWP]
    Apv = A[:, :, DATA_W0 - 1:NWP - 1]   # prev word (word 7 = pad zero)
    nc.vector.tensor_tensor(out=t1[:], in0=Ad, in1=r8b, op=op.logical_shift_left)
    nc.vector.tensor_tensor(out=t2[:], in0=Apv, in1=invb,
                            op=op.logical_shift_right)
    nc.vector.tensor_scalar(out=t2[:], in0=t2[:], scalar1=1, scalar2=None,
                            op0=op.logical_shift_right)
    nc.vector.tensor_tensor(out=B[:, :, DATA_W0:NWP], in0=t1[:], in1=t2[:],
                            op=op.bitwise_or)
    co = pool.tile([P, C], dt.uint32, tag="co")
    nc.vector.tensor_tensor(out=co[:], in0=A[:, :, NWP - 1], in1=inv[:],
                            op=op.logical_shift_right)
    nc.vector.tensor_scalar(out=acc[:], in0=co[:], scalar1=1, scalar2=None,
                            op0=op.logical_shift_right)

    # word-shift mux layers, in place on B with descending (reversed) APs:
    # writes land on positions already past the read cursor, so reads see
    # pre-instruction values (equivalent to an atomic mux).
    for (amt, col) in ((1, 3), (2, 2), (4, 1), (8, 0)):
        mask1 = shfi[:, :, col:col + 1].squeeze(2)
        maskb = shfi[:, :, col:col + 1].to_broadcast([P, C, NW])
        cap = pool.tile([P, C], dt.uint32, tag="cap")
        if amt == 1:
            nc.vector.tensor_copy(cap[:], B[:, :, NWP - 1])
        else:
            nc.vector.tensor_reduce(out=cap[:], in_=B[:, :, NWP - amt:NWP],
                                    axis=mybir.AxisListType.X, op=op.max)
        tmax = pool.tile([P, C], dt.uint32, tag="tmax")
        nc.vector.tensor_tensor(out=tmax[:], in0=acc[:], in1=cap[:], op=op.max)
        nc.vector.copy_predicated(out=acc[:], mask=mask1, data=tmax[:])
        dstop = DATA_W0 - 1 - amt
        nc.vector.copy_predicated(
            out=B[:, :, NWP - 1:DATA_W0 - 1:-1], mask=maskb,
            data=B[:, :, NWP - 1 - amt:(dstop if dstop >= 0 else None):-1])

    fin = B
    fin8 = fin.bitcast(dt.uint8)

    # final sticky: bytes 57..63 of the final value OR carry accum
    tl = pool.tile([P, C], dt.uint32, tag="tl")
    nc.vector.tensor_scalar(out=tl[:], in0=fin[:, :, NWP - 2], scalar1=8,
                            scalar2=None, op0=op.logical_shift_right)
    nc.vector.tensor_tensor(out=tl[:], in0=tl[:], in1=fin[:, :, NWP - 1],
                            op=op.max)
    nc.vector.tensor_tensor(out=tl[:], in0=tl[:], in1=acc[:], op=op.max)
    stk = pool.tile([P, C], dt.float32, tag="stk")
    nc.vector.tensor_scalar(out=stk[:], in0=tl[:], scalar1=0, scalar2=None,
                            op0=op.not_equal)

    # cast u8->f32 on ACT, contiguous HWDGE store
    yf = pool.tile([P, C, DATA_BITS], dt.float32, tag="yf")
    nc.scalar.copy(out=yf[:], in_=fin8[:, :, DATA_B0:DATA_B0 + DATA_BITS])
    nc.sync.dma_start(out=out_rows, in_=yf[:])
    nc.sync.dma_start(out=stk_rows, in_=stk[:].unsqueeze(2))


def _build_program():
    nc = bacc.Bacc("TRN2", target_bir_lowering=False, debug=False)
    x = nc.dram_tensor("X", [B_PAD, DATA_BITS], mybir.dt.float32,
                       kind="ExternalInput").ap()
    sh = nc.dram_tensor("shift", [B_PAD, SHIFT_BITS], mybir.dt.float32,
                        kind="ExternalInput").ap()
    out = nc.dram_tensor("out", [B_PAD, DATA_BITS], mybir.dt.float32,
                         kind="ExternalOutput").ap()
    stk = nc.dram_tensor("stk", [B_PAD, 1], mybir.dt.float32,
                         kind="ExternalOutput").ap()
    with tile.TileContext(nc) as tc:
        with tc.tile_pool(name="pool", bufs=2) as pool, \
             tc.tile_pool(name="pool1", bufs=1) as pool1:
            for (b0, C) in TILE_PLAN:
                n = P * C
                xr = x[b0:b0 + n, :].rearrange("(p c) j -> p c j", c=C)
                sr = sh[b0:b0 + n, :].rearrange("(p c) j -> p c j", c=C)
                orr = out[b0:b0 + n, :].rearrange("(p c) j -> p c j", c=C)
                kr = stk[b0:b0 + n, :].rearrange("(p c) j -> p c j", c=C)
                _barrel_tile(nc, pool, pool1, xr, sr, orr, kr, C)
    nc.compile()
    return nc


_PROGRAM = None


def _get_program():
    global _PROGRAM
    if _PROGRAM is None:
        _PROGRAM = _build_program()
    return _PROGRAM


def run(X, shift, trace=False):
    """Run on 8 NeuronCores. Returns ((result, sticky), BassKernelResults)."""
    X = np.ascontiguousarray(np.asarray(X, dtype=np.float32))
    shift = np.ascontiguousarray(np.asarray(shift, dtype=np.float32))
    assert X.shape == (B_TOTAL, DATA_BITS), X.shape
    assert shift.shape == (B_TOTAL, SHIFT_BITS), shift.shape
    nc = _get_program()
    in_maps = []
    for i in range(N_CORES):
        xp = np.zeros((B_PAD, DATA_BITS), np.float32)
        sp = np.zeros((B_PAD, SHIFT_BITS), np.float32)
        xp[:B_CORE] = X[i * B_CORE:(i + 1) * B_CORE]
        sp[:B_CORE] = shift[i * B_CORE:(i + 1) * B_CORE]
        in_maps.append({"X": xp, "shift": sp})
    res = run_bass_kernel_spmd(nc, in_maps, core_ids=list(range(N_CORES)),
                               trace=trace)
    outs = np.concatenate([r["out"][:B_CORE] for r in res.results], axis=0)
    stks = np.concatenate([r["stk"][:B_CORE] for r in res.results], axis=0)
    return (outs, stks), res


def kernel(X, shift):
    (outs, stks), _ = run(X, shift)
    return outs, stks
